# revision 21
# baseline (speedup 1.0000x reference)
"""BitLinear fake-quant GEMM on 8 trn2 NeuronCores, data-parallel over batch.

Per core: y[s,o] = round(clip(x/a_scale*127)) @ clip(round(w/w_scale),-1,1)^T
          * (w_scale * a_scale / 127),  a_scale = rowmax|x| + eps.

Quantized activations are integers |v|<=127 and weights are ternary, so a
bf16 matmul with fp32 PSUM accumulation is exact integer arithmetic. Weight
ternarization is done exactly (f32, round-half-even) on the host and shipped
as bf16; x is shipped bf16 (quantization flips stay far inside the output
tolerance) and y is stored bf16, halving both streams' HBM traffic.

Key hardware constraint this layout works around: every xbar DMA-transpose
is serialized against ALL other DMAs (it waits for every prior-scheduled DMA
to complete, and every later DMA waits for it). So transposes are batched
into 4-tile groups, per-slot no_sync_barriers pin the schedule so a
transpose never serializes against a far-future x load, and in-slot DMA
order is transpose -> loads -> stores.

Engine plan per pair-slot (PE paces at ~7us/pair):
  POOL  x pair-load (SWDGE)      ACT  q1 = x*rec127+RND, then prev epis
  DVE   q2 = tq-RND -> bf16 aq, then rowmax stats
  SP    grouped xbar transpose aq->aqT, then y pair-store
  PE    32 matmuls [K=128 x N=512]
"""

import os
import sys

import numpy as np

sys.path.insert(0, "/opt/trn_rl_repo")

import concourse.bacc as bacc
import concourse.mybir as mybir
import concourse.tile as tile
from concourse.bass_utils import run_bass_kernel_spmd

F32 = mybir.dt.float32
BF16 = mybir.dt.bfloat16
AF = mybir.ActivationFunctionType
ALU = mybir.AluOpType

B = 8
S = 4096
D = 1024
O = 1024
P = 128
KB = D // P
RND = 12582912.0  # 1.5*2**23: (z+RND)-RND == round-half-even(z) for |z|<2**22
EPS = 1e-8

# transpose groups (tiles per xbar transpose); ramp small, steady 4-tile
GROUPS = [[0], [1], [2, 3], [4, 5, 6, 7], [8, 9, 10, 11], [12, 13, 14, 15],
          [16, 17, 18, 19], [20, 21, 22, 23], [24, 25, 26, 27],
          [28, 29, 30, 31]]

_CACHE = {}
TRACE_DIR = None


def _build(s_rows=S):
    nt = s_rows // P
    np_ = nt // 2
    group_of = {}
    for gi, g in enumerate(GROUPS):
        for local, t in enumerate(g):
            group_of[t] = (gi, local)

    nc = bacc.Bacc("TRN2", target_bir_lowering=False, debug=False)
    x_d = nc.dram_tensor("x", [s_rows, D], BF16, kind="ExternalInput")
    w_d = nc.dram_tensor("wq", [D, O], BF16, kind="ExternalInput")
    ws_d = nc.dram_tensor("wsb", [P, 1], F32, kind="ExternalInput")
    y_d = nc.dram_tensor("y", [s_rows, O], BF16, kind="ExternalOutput")
    xa, wa, wsa, ya = x_d.ap(), w_d.ap(), ws_d.ap(), y_d.ap()
    xp = xa.rearrange("(q a p) d -> q p a d", p=P, a=2)
    yp = ya.rearrange("(q a p) o -> q p a o", p=P, a=2)

    with tile.TileContext(nc) as tc:
        with (
            tc.tile_pool(name="wq", bufs=1) as wq_p,
            tc.tile_pool(name="xg", bufs=6) as xg_p,
            tc.tile_pool(name="stat", bufs=17) as stat_p,
            tc.tile_pool(name="tq", bufs=6) as tq_p,
            tc.tile_pool(name="aq", bufs=2) as aq_p,
            tc.tile_pool(name="aqT", bufs=3) as aqT_p,
            tc.tile_pool(name="yout", bufs=4) as y_p,
            tc.tile_pool(name="psum", bufs=4, space="PSUM") as ps_p,
        ):
            wsb = stat_p.tile([P, 1], F32, tag="wsb", bufs=1)
            nc.scalar.dma_start(out=wsb[:], in_=wsa[:, :])
            wq = wq_p.tile([P, KB, O], BF16)
            wv = wa.rearrange("(k p) o -> p k o", p=P)
            nc.scalar.dma_start(out=wq[:, :, 0:512], in_=wv[:, :, 0:512])
            nc.scalar.dma_start(out=wq[:, :, 512:1024], in_=wv[:, :, 512:1024])

            xts, stats, tqs, aqs, aqTs, psums, ysbs = {}, {}, {}, {}, {}, {}, {}

            def load_pair(q):
                if not (0 <= q < np_):
                    return
                xt = xg_p.tile([P, 2, D], BF16, tag="xt", name="xt")
                nc.gpsimd.dma_start(out=xt[:], in_=xp[q])
                xts[q] = xt

            def load_tile(t):  # startup: tile-granular halves of pair 0
                q, i = t // 2, t % 2
                if q not in xts:
                    xts[q] = xg_p.tile([P, 2, D], BF16, tag="xt", name="xt")
                nc.gpsimd.dma_start(out=xts[q][:, i, :],
                                    in_=xa[t * P:(t + 1) * P, :])

            def emit_stats(q, i=None):
                if not (0 <= q < np_):
                    return
                if q not in stats:
                    stats[q] = (
                        stat_p.tile([P, 2], F32, tag="st", name="st"),
                        stat_p.tile([P, 2], F32, tag="ga2", name="ga2"),
                        stat_p.tile([P, 2], F32, tag="rec", name="rec"),
                        stat_p.tile([P, 2], F32, tag="epi", name="epi"),
                    )
                st, ga2, rec, epi = stats[q]
                sl = slice(None) if i is None else slice(i, i + 1)
                src = xts[q][:] if i is None else xts[q][:, i:i + 1, :]
                nc.vector.tensor_reduce(st[:, sl], src, mybir.AxisListType.X,
                                        ALU.max, apply_absolute_value=True)
                nc.vector.tensor_scalar(ga2[:, sl], st[:, sl], EPS, 1.0 / 127.0,
                                        ALU.add, ALU.mult)
                nc.vector.reciprocal(rec[:, sl], ga2[:, sl])
                nc.vector.tensor_scalar(epi[:, sl], ga2[:, sl], wsb[:], None,
                                        ALU.mult)

            def emit_quant(t):
                # ACT: tq = x*rec127 + RND ; DVE: aq = tq - RND -> bf16
                if not (0 <= t < nt):
                    return
                q, i = t // 2, t % 2
                gi, local = group_of[t]
                rec = stats[q][2]
                tq = tq_p.tile([P, D], F32, tag="tq", name="tq")
                nc.scalar.activation(tq[:], xts[q][:, i, :], AF.Copy,
                                     bias=RND, scale=rec[:, i:i + 1])
                if gi not in aqs:
                    n = len(GROUPS[gi])
                    aqs[gi] = aq_p.tile([P, n, D], BF16, tag=f"aq{n}",
                                        name="aq")
                nc.vector.tensor_scalar(aqs[gi][:, local, :], tq[:], RND, None,
                                        ALU.subtract)

            def emit_transpose(gi):
                n = len(GROUPS[gi])
                aqTs[gi] = aqT_p.tile([P, n * KB, P], BF16, tag=f"aqT{n}",
                                      name="aqT")
                nc.sync.dma_start_transpose(
                    aqTs[gi][:], aqs[gi].rearrange("p a d -> p (a d)"))

            def emit_mms_tile(t):
                if not (0 <= t < nt):
                    return
                gi, local = group_of[t]
                aqT = aqTs[gi]
                yt = ps_p.tile([P, O], F32, name="yt")
                for bank in range(2):
                    o0 = bank * 512
                    for k in range(KB):
                        nc.tensor.matmul(
                            yt[:, o0:o0 + 512], aqT[:, local * KB + k, :],
                            wq[:, k, o0:o0 + 512],
                            start=(k == 0), stop=(k == KB - 1))
                psums[t] = yt

            def emit_epis(q):
                # ACT: ysb = psum * (a_scale*ws/127) -> bf16, pair tile
                if not (0 <= q < np_):
                    return
                epi = stats[q][3]
                ysb = y_p.tile([P, 2, O], BF16, tag="ysb", name="ysb")
                for i in range(2):
                    nc.scalar.activation(ysb[:, i, :], psums.pop(2 * q + i)[:],
                                         AF.Copy, bias=0.0,
                                         scale=epi[:, i:i + 1])
                ysbs[q] = ysb

            def emit_store(q):
                if not (0 <= q < np_):
                    return
                nc.gpsimd.dma_start(out=yp[q], in_=ysbs.pop(q)[:])

            # ---------------- prologue ----------------
            load_tile(0)
            load_tile(1)
            emit_stats(0, i=0)
            emit_quant(0)
            emit_transpose(0)          # [t0]
            emit_stats(0, i=1)
            emit_quant(1)
            emit_transpose(1)          # [t1]
            emit_mms_tile(0)
            emit_mms_tile(1)
            for q in (1, 2, 3):
                load_pair(q)
            emit_stats(1)
            emit_stats(2)
            emit_stats(3)
            emit_quant(2)
            emit_quant(3)
            emit_transpose(2)          # [t2,t3]
            for t in (4, 5, 6, 7):
                emit_quant(t)
            emit_transpose(3)          # [t4..t7]
            tc.no_sync_barrier()
            for q in (4, 5, 6, 7):
                load_pair(q)
            emit_stats(4)
            emit_stats(5)
            tc.no_sync_barrier()

            # ---------------- steady slots ----------------
            quant_sched = {1: [8, 9, 10, 11], 2: [12, 13, 14, 15],
                           3: [16, 17], 4: [18, 19], 5: [20, 21], 6: [22, 23],
                           7: [24, 25], 8: [26, 27], 9: [28, 29], 10: [30, 31]}
            trans_sched = {1: 4, 2: 5, 4: 6, 6: 7, 8: 8, 10: 9}
            load_sched = {1: [8, 9], 2: [10, 11], 3: [12], 4: [13], 5: [14],
                          6: [15]}
            stats_sched = {1: [6, 7], 2: [8, 9], 3: [10, 11], 5: [12, 13],
                           7: [14, 15]}
            for p in range(1, np_ + 2):
                for t in quant_sched.get(p, []):
                    emit_quant(t)
                if p in trans_sched:
                    emit_transpose(trans_sched[p])
                for q in load_sched.get(p, []):
                    load_pair(q)
                for q in stats_sched.get(p, []):
                    emit_stats(q)
                if p < np_:
                    emit_mms_tile(2 * p)
                    emit_mms_tile(2 * p + 1)
                emit_epis(p - 1)
                emit_store(p - 2)
                tc.no_sync_barrier()
    nc.compile()
    return nc


def _prep_weight(weight):
    # Mirror the reference exactly in f32: w_scale = mean|w|+eps (f64 mean
    # rounded to f32 like any fp32 summation order allows), u = w/ws in f32,
    # ternary = clip(round-half-even(u), -1, 1). Ternary values are exact in
    # bf16; ws is folded into the epilogue scale (epi = (max+eps)/127 * ws).
    m = np.abs(weight.astype(np.float64)).mean()
    ws = np.float32(np.float32(m) + np.float32(EPS))
    u = (weight / ws).astype(np.float32)
    wq = np.clip(np.round(u), -1.0, 1.0).astype(np.float32)
    import ml_dtypes
    wqT = np.ascontiguousarray(wq.T).astype(ml_dtypes.bfloat16)
    wsb = np.full((P, 1), ws, dtype=np.float32)
    return wqT, wsb


def kernel(x, weight):
    import ml_dtypes
    x = np.ascontiguousarray(np.asarray(x)).astype(ml_dtypes.bfloat16)
    weight = np.ascontiguousarray(np.asarray(weight), dtype=np.float32)
    assert x.shape == (B, S, D) and weight.shape == (O, D)
    nc = _CACHE.get("nc")
    if nc is None:
        nc = _CACHE["nc"] = _build()
    wqT, wsb = _prep_weight(weight)
    in_maps = [{"x": x[c], "wq": wqT, "wsb": wsb} for c in range(B)]
    trace = bool(int(os.environ.get("BITLINEAR_TRACE", "0")))
    res = run_bass_kernel_spmd(
        nc, in_maps, list(range(B)), trace=trace, tmpdir=TRACE_DIR
    )
    _CACHE["last"] = res
    return np.stack(
        [np.asarray(res.results[c]["y"]).astype(np.float32) for c in range(B)],
        axis=0,
    )


# revision 22
# speedup vs baseline: 1.0088x; 1.0088x over previous
"""BitLinear fake-quant GEMM on 8 trn2 NeuronCores, data-parallel over batch.

Per core: y[s,o] = round(clip(x/a_scale*127)) @ clip(round(w/w_scale),-1,1)^T
          * (w_scale * a_scale / 127),  a_scale = rowmax|x| + eps.

Quantized activations are integers |v|<=127 and weights are ternary, so a
bf16 matmul with fp32 PSUM accumulation is exact integer arithmetic. Weight
ternarization is done exactly (f32, round-half-even) on the host and shipped
as bf16; x is shipped bf16 (quantization flips stay far inside the output
tolerance) and y is stored bf16, halving both streams' HBM traffic.

Key hardware constraint this layout works around: every xbar DMA-transpose
is serialized against ALL other DMAs (it waits for every prior-scheduled DMA
to complete, and every later DMA waits for it). So transposes are batched
into 4-tile groups, per-slot no_sync_barriers pin the schedule so a
transpose never serializes against a far-future x load, and in-slot DMA
order is transpose -> loads -> stores.

Engine plan per pair-slot (PE paces at ~7us/pair):
  POOL  x pair-load (SWDGE)      ACT  q1 = x*rec127+RND, then prev epis
  DVE   q2 = tq-RND -> bf16 aq, then rowmax stats
  SP    grouped xbar transpose aq->aqT, then y pair-store
  PE    32 matmuls [K=128 x N=512]
"""

import os
import sys

import numpy as np

sys.path.insert(0, "/opt/trn_rl_repo")

import concourse.bacc as bacc
import concourse.mybir as mybir
import concourse.tile as tile
from concourse.bass_utils import run_bass_kernel_spmd

F32 = mybir.dt.float32
BF16 = mybir.dt.bfloat16
AF = mybir.ActivationFunctionType
ALU = mybir.AluOpType

B = 8
S = 4096
D = 1024
O = 1024
P = 128
KB = D // P
RND = 12582912.0  # 1.5*2**23: (z+RND)-RND == round-half-even(z) for |z|<2**22
EPS = 1e-8

# transpose groups (tiles per xbar transpose); ramp small, steady 4-tile
GROUPS = [[0], [1], [2, 3], [4, 5, 6, 7], [8, 9, 10, 11], [12, 13, 14, 15],
          [16, 17, 18, 19], [20, 21, 22, 23], [24, 25, 26, 27],
          [28, 29, 30, 31]]

_CACHE = {}
TRACE_DIR = None


def _build(s_rows=S):
    nt = s_rows // P
    np_ = nt // 2
    group_of = {}
    for gi, g in enumerate(GROUPS):
        for local, t in enumerate(g):
            group_of[t] = (gi, local)

    nc = bacc.Bacc("TRN2", target_bir_lowering=False, debug=False)
    x_d = nc.dram_tensor("x", [s_rows, D], BF16, kind="ExternalInput")
    w_d = nc.dram_tensor("wq", [D, O], BF16, kind="ExternalInput")
    ws_d = nc.dram_tensor("wsb", [P, 1], F32, kind="ExternalInput")
    y_d = nc.dram_tensor("y", [s_rows, O], BF16, kind="ExternalOutput")
    xa, wa, wsa, ya = x_d.ap(), w_d.ap(), ws_d.ap(), y_d.ap()
    xp = xa.rearrange("(q a p) d -> q p a d", p=P, a=2)
    yp = ya.rearrange("(q a p) o -> q p a o", p=P, a=2)

    with tile.TileContext(nc) as tc:
        with (
            tc.tile_pool(name="wq", bufs=1) as wq_p,
            tc.tile_pool(name="xg", bufs=6) as xg_p,
            tc.tile_pool(name="stat", bufs=17) as stat_p,
            tc.tile_pool(name="tq", bufs=6) as tq_p,
            tc.tile_pool(name="aq", bufs=2) as aq_p,
            tc.tile_pool(name="aqT", bufs=3) as aqT_p,
            tc.tile_pool(name="yout", bufs=4) as y_p,
            tc.tile_pool(name="psum", bufs=4, space="PSUM") as ps_p,
        ):
            wsb = stat_p.tile([P, 1], F32, tag="wsb", bufs=1)
            nc.scalar.dma_start(out=wsb[:], in_=wsa[:, :])
            wq = wq_p.tile([P, KB, O], BF16)
            wv = wa.rearrange("(k p) o -> p k o", p=P)
            nc.scalar.dma_start(out=wq[:, :, 0:512], in_=wv[:, :, 0:512])
            nc.scalar.dma_start(out=wq[:, :, 512:1024], in_=wv[:, :, 512:1024])

            xts, stats, tqs, aqs, aqTs, psums, ysbs = {}, {}, {}, {}, {}, {}, {}

            # HAM warm-up: ~45 throwaway matmuls on zeroed scratch so the PE
            # clock-gate is released (K=8/8) before the first real matmul
            # arrives (~21us). They overwrite psum bank 0 of the first real
            # tile (start=True each; the real group re-clears it).
            warm = tq_p.tile([P, 512], BF16, tag="warm", name="warm", bufs=1)
            nc.vector.memset(warm[:], 0)
            yt0 = ps_p.tile([P, O], F32, name="yt")
            for _ in range(45):
                nc.tensor.matmul(yt0[:, 0:512], warm[:, 0:128], warm[:],
                                 start=True, stop=True, skip_group_check=True)

            def load_pair(q):
                if not (0 <= q < np_):
                    return
                xt = xg_p.tile([P, 2, D], BF16, tag="xt", name="xt")
                nc.gpsimd.dma_start(out=xt[:], in_=xp[q])
                xts[q] = xt

            def load_tile(t):  # startup: tile-granular halves of pair 0
                q, i = t // 2, t % 2
                if q not in xts:
                    xts[q] = xg_p.tile([P, 2, D], BF16, tag="xt", name="xt")
                nc.gpsimd.dma_start(out=xts[q][:, i, :],
                                    in_=xa[t * P:(t + 1) * P, :])

            def emit_stats(q, i=None):
                if not (0 <= q < np_):
                    return
                if q not in stats:
                    stats[q] = (
                        stat_p.tile([P, 2], F32, tag="st", name="st"),
                        stat_p.tile([P, 2], F32, tag="ga2", name="ga2"),
                        stat_p.tile([P, 2], F32, tag="rec", name="rec"),
                        stat_p.tile([P, 2], F32, tag="epi", name="epi"),
                    )
                st, ga2, rec, epi = stats[q]
                sl = slice(None) if i is None else slice(i, i + 1)
                src = xts[q][:] if i is None else xts[q][:, i:i + 1, :]
                nc.vector.tensor_reduce(st[:, sl], src, mybir.AxisListType.X,
                                        ALU.max, apply_absolute_value=True)
                nc.vector.tensor_scalar(ga2[:, sl], st[:, sl], EPS, 1.0 / 127.0,
                                        ALU.add, ALU.mult)
                nc.vector.reciprocal(rec[:, sl], ga2[:, sl])
                nc.vector.tensor_scalar(epi[:, sl], ga2[:, sl], wsb[:], None,
                                        ALU.mult)

            def emit_quant(t):
                # ACT: tq = x*rec127 + RND ; DVE: aq = tq - RND -> bf16
                if not (0 <= t < nt):
                    return
                q, i = t // 2, t % 2
                gi, local = group_of[t]
                rec = stats[q][2]
                tq = tq_p.tile([P, D], F32, tag="tq", name="tq")
                nc.scalar.activation(tq[:], xts[q][:, i, :], AF.Copy,
                                     bias=RND, scale=rec[:, i:i + 1])
                if gi not in aqs:
                    n = len(GROUPS[gi])
                    aqs[gi] = aq_p.tile([P, n, D], BF16, tag=f"aq{n}",
                                        name="aq")
                nc.vector.tensor_scalar(aqs[gi][:, local, :], tq[:], RND, None,
                                        ALU.subtract)

            def emit_transpose(gi):
                n = len(GROUPS[gi])
                aqTs[gi] = aqT_p.tile([P, n * KB, P], BF16, tag=f"aqT{n}",
                                      name="aqT")
                nc.sync.dma_start_transpose(
                    aqTs[gi][:], aqs[gi].rearrange("p a d -> p (a d)"))

            def emit_mms_tile(t):
                if not (0 <= t < nt):
                    return
                gi, local = group_of[t]
                aqT = aqTs[gi]
                yt = yt0 if t == 0 else ps_p.tile([P, O], F32, name="yt")
                for bank in range(2):
                    o0 = bank * 512
                    for k in range(KB):
                        nc.tensor.matmul(
                            yt[:, o0:o0 + 512], aqT[:, local * KB + k, :],
                            wq[:, k, o0:o0 + 512],
                            start=(k == 0), stop=(k == KB - 1))
                psums[t] = yt

            def emit_epis(q):
                # ACT: ysb = psum * (a_scale*ws/127) -> bf16, pair tile.
                # Last pair: split across DVE+ACT so the tail drains faster.
                if not (0 <= q < np_):
                    return
                epi = stats[q][3]
                ysb = y_p.tile([P, 2, O], BF16, tag="ysb", name="ysb")
                if q == np_ - 1:
                    nc.vector.tensor_scalar(ysb[:, 0, :], psums.pop(2 * q)[:],
                                            epi[:, 0:1], None, ALU.mult)
                    nc.scalar.activation(ysb[:, 1, :], psums.pop(2 * q + 1)[:],
                                         AF.Copy, bias=0.0, scale=epi[:, 1:2])
                else:
                    for i in range(2):
                        nc.scalar.activation(ysb[:, i, :],
                                             psums.pop(2 * q + i)[:],
                                             AF.Copy, bias=0.0,
                                             scale=epi[:, i:i + 1])
                ysbs[q] = ysb

            def emit_store(q):
                if not (0 <= q < np_):
                    return
                nc.sync.dma_start(out=yp[q], in_=ysbs.pop(q)[:])

            # ---------------- prologue ----------------
            load_tile(0)
            load_tile(1)
            emit_stats(0, i=0)
            emit_quant(0)
            emit_transpose(0)          # [t0]
            emit_stats(0, i=1)
            emit_quant(1)
            emit_transpose(1)          # [t1]
            emit_mms_tile(0)
            emit_mms_tile(1)
            for q in (1, 2, 3):
                load_pair(q)
            emit_stats(1)
            emit_stats(2)
            emit_stats(3)
            emit_quant(2)
            emit_quant(3)
            emit_transpose(2)          # [t2,t3]
            for t in (4, 5, 6, 7):
                emit_quant(t)
            emit_transpose(3)          # [t4..t7]
            tc.no_sync_barrier()
            for q in (4, 5, 6, 7):
                load_pair(q)
            emit_stats(4)
            emit_stats(5)
            tc.no_sync_barrier()

            # ---------------- steady slots ----------------
            quant_sched = {1: [8, 9, 10, 11], 2: [12, 13, 14, 15],
                           3: [16, 17], 4: [18, 19], 5: [20, 21], 6: [22, 23],
                           7: [24, 25], 8: [26, 27], 9: [28, 29], 10: [30, 31]}
            trans_sched = {1: 4, 2: 5, 4: 6, 6: 7, 8: 8, 10: 9}
            load_sched = {1: [8, 9], 2: [10, 11], 3: [12], 4: [13], 5: [14],
                          6: [15]}
            stats_sched = {1: [6, 7], 2: [8, 9], 3: [10, 11], 5: [12, 13],
                           7: [14, 15]}
            for p in range(1, np_ + 2):
                for t in quant_sched.get(p, []):
                    emit_quant(t)
                if p in trans_sched:
                    emit_transpose(trans_sched[p])
                for q in load_sched.get(p, []):
                    load_pair(q)
                for q in stats_sched.get(p, []):
                    emit_stats(q)
                if p < np_:
                    emit_mms_tile(2 * p)
                    emit_mms_tile(2 * p + 1)
                emit_epis(p - 1)
                emit_store(p - 2)
                tc.no_sync_barrier()
    nc.compile()
    return nc


def _prep_weight(weight):
    # Mirror the reference exactly in f32: w_scale = mean|w|+eps (f64 mean
    # rounded to f32 like any fp32 summation order allows), u = w/ws in f32,
    # ternary = clip(round-half-even(u), -1, 1). Ternary values are exact in
    # bf16; ws is folded into the epilogue scale (epi = (max+eps)/127 * ws).
    m = np.abs(weight.astype(np.float64)).mean()
    ws = np.float32(np.float32(m) + np.float32(EPS))
    u = (weight / ws).astype(np.float32)
    wq = np.clip(np.round(u), -1.0, 1.0).astype(np.float32)
    import ml_dtypes
    wqT = np.ascontiguousarray(wq.T).astype(ml_dtypes.bfloat16)
    wsb = np.full((P, 1), ws, dtype=np.float32)
    return wqT, wsb


def kernel(x, weight):
    import ml_dtypes
    x = np.ascontiguousarray(np.asarray(x)).astype(ml_dtypes.bfloat16)
    weight = np.ascontiguousarray(np.asarray(weight), dtype=np.float32)
    assert x.shape == (B, S, D) and weight.shape == (O, D)
    nc = _CACHE.get("nc")
    if nc is None:
        nc = _CACHE["nc"] = _build()
    wqT, wsb = _prep_weight(weight)
    in_maps = [{"x": x[c], "wq": wqT, "wsb": wsb} for c in range(B)]
    trace = bool(int(os.environ.get("BITLINEAR_TRACE", "0")))
    res = run_bass_kernel_spmd(
        nc, in_maps, list(range(B)), trace=trace, tmpdir=TRACE_DIR
    )
    _CACHE["last"] = res
    return np.stack(
        [np.asarray(res.results[c]["y"]).astype(np.float32) for c in range(B)],
        axis=0,
    )
